# revision 1
# baseline (speedup 1.0000x reference)
"""Complex Conv1D (VALID, stride 1) on Trainium2 — Bass/Tile, 8-core data-parallel.

Problem (hardcoded shapes):
  x_real/x_imag: [32, 4096, 64] f32, kernel_real/imag: [9, 64, 64] f32,
  bias_real/imag: [64] f32  ->  out [32, 4088, 64, 2] f32
  out_real = conv(xr, wr) - conv(xi, wi) + br
  out_imag = conv(xr, wi) + conv(xi, wr) + bi

Mapping: complex multiply as its 2x2 real block-matrix form so each tap is ONE
full 128-contract matmul:
  X_b [128, L]   rows 0:64 = xr[b].T (channels on partitions), 64:128 = xi[b].T
  W[k] [128,128] = [[wr[k], wi[k]], [-wi[k], wr[k]]]
  psum[128, T] += W[k].T @ X_b[:, l0+k : l0+k+T]   for k = 0..8
  psum rows 0:64 = real output (filters), rows 64:128 = imag output.
Batch is sharded 4-per-core across 8 cores; weights replicated. The kernel
emits the output transposed as [b, 128, L_out]; the host restores
[B, L_out, F, 2].
"""

import numpy as np

import concourse.bacc as bacc
import concourse.bass as bass
import concourse.mybir as mybir
from concourse.tile import TileContext
from concourse.bass_utils import run_bass_kernel_spmd

B, L, CIN, KT, F = 32, 4096, 64, 9, 64
LOUT = L - KT + 1  # 4088
NCORES = 8
BPC = B // NCORES  # batches per core
TL = 512  # output-tile width (one PSUM bank of fp32)
NLT = (LOUT + TL - 1) // TL  # 8

# Matmul operand dtype: float32r streams fp32 operands through the PE in a
# single reduced-precision pass (full rate for N>=256); plain float32 is the
# exact-but-4x-slower fallback; bfloat16 halves DMA traffic.
MM_DT_NAME = "float32r"


def _build_nc(
    mm_dt,
    w_dt=None,
    xbufs=3,
    obufs=4,
    psbufs=4,
    warmup=0,
    evac="act",
    repeat=1,
    loop_repeat=None,
):
    nc = bacc.Bacc("TRN2", target_bir_lowering=False, debug=False, num_devices=NCORES)
    if w_dt is None:
        w_dt = mm_dt

    x_d = nc.dram_tensor("x", [BPC, 128, L], mm_dt, kind="ExternalInput")
    w_d = nc.dram_tensor("w", [128, KT * 128], w_dt, kind="ExternalInput")
    bias_d = nc.dram_tensor("bias", [128, 1], mybir.dt.float32, kind="ExternalInput")
    out_d = nc.dram_tensor("out", [BPC, 128, LOUT], mybir.dt.float32, kind="ExternalOutput")

    f32 = mybir.dt.float32
    ident = mybir.ActivationFunctionType.Identity

    with TileContext(nc) as tc:
        with (
            tc.tile_pool(name="wpool", bufs=1) as wpool,
            tc.tile_pool(name="xpool", bufs=xbufs) as xpool,
            tc.tile_pool(name="opool", bufs=obufs) as opool,
            tc.tile_pool(name="pspool", bufs=psbufs, space="PSUM") as pspool,
        ):
            wt = wpool.tile([128, KT * 128], w_dt)
            nc.sync.dma_start(wt[:], w_d[:])
            bias_t = wpool.tile([128, 1], f32)
            nc.sync.dma_start(bias_t[:], bias_d[:])

            if warmup:
                # Keep the PE busy (HAM ramp) while the first x tiles load.
                wps = pspool.tile([128, 128], f32, tag="wps", bufs=1)
                for i in range(warmup):
                    nc.tensor.matmul(
                        wps[:], wt[:, 0:128], wt[:, 0:128],
                        start=True, stop=True, skip_group_check=True,
                    )

            import contextlib

            loop_cm = (
                tc.For_i(0, loop_repeat, 1)
                if loop_repeat is not None
                else contextlib.nullcontext()
            )
            n_evac = 0
            with loop_cm:
              for _rep in range(repeat):
                for b in range(BPC):
                    for j in range(NLT):
                        l0 = j * TL
                        t = min(TL, LOUT - l0)
                        w_in = min(L, l0 + t + KT - 1) - l0
                        xt = xpool.tile([128, TL + KT - 1], mm_dt, tag="xt")
                        nc.sync.dma_start(xt[:, :w_in], x_d[b, :, l0 : l0 + w_in])
                        ps = pspool.tile([128, TL], f32, tag="ps")
                        for k in range(KT):
                            nc.tensor.matmul(
                                ps[:, :t],
                                wt[:, k * 128 : (k + 1) * 128],
                                xt[:, k : k + t],
                                start=(k == 0),
                                stop=(k == KT - 1),
                            )
                        ot = opool.tile([128, TL], f32, tag="ot")
                        if evac == "alt" and n_evac % 2 == 1:
                            nc.vector.tensor_scalar_add(
                                ot[:, :t], ps[:, :t], bias_t[:]
                            )
                        else:
                            nc.scalar.activation(
                                ot[:, :t], ps[:, :t], ident, bias=bias_t[:]
                            )
                        n_evac += 1
                        nc.sync.dma_start(out_d[b, :, l0 : l0 + t], ot[:, :t])

    nc.compile()
    return nc


def _pack(x_real, x_imag, kernel_real, kernel_imag, bias_real, bias_imag, np_dt,
          w_np_dt=None):
    if w_np_dt is None:
        w_np_dt = np_dt
    X = np.empty((B, 128, L), np_dt)
    X[:, :CIN] = x_real.transpose(0, 2, 1)
    X[:, CIN:] = x_imag.transpose(0, 2, 1)
    Wk = np.empty((KT, 128, 128), np.float32)
    Wk[:, :CIN, :F] = kernel_real
    Wk[:, :CIN, F:] = kernel_imag
    Wk[:, CIN:, :F] = -kernel_imag
    Wk[:, CIN:, F:] = kernel_real
    W2 = Wk.transpose(1, 0, 2).reshape(128, KT * 128).astype(w_np_dt)
    bias2 = (
        np.concatenate([bias_real, bias_imag]).reshape(128, 1).astype(np.float32)
    )
    return X, np.ascontiguousarray(W2), bias2


def _parse_dt(name):
    name = name or MM_DT_NAME
    if "," in name:
        xn, wn = name.split(",")
    else:
        xn = wn = name
    return getattr(mybir.dt, xn), getattr(mybir.dt, wn)


def _prepare(inputs, mm_dt_name=None, build_kw=None):
    mm_dt, w_dt = _parse_dt(mm_dt_name)
    np_dt = mybir.dt.np(mm_dt)
    w_np_dt = mybir.dt.np(w_dt)
    args = {
        k: np.asarray(inputs[k], np.float32)
        for k in (
            "x_real", "x_imag", "kernel_real", "kernel_imag", "bias_real", "bias_imag",
        )
    }
    X, W2, bias2 = _pack(np_dt=np_dt, w_np_dt=w_np_dt, **args)

    nc = _build_nc(mm_dt, w_dt=w_dt, **(build_kw or {}))
    in_maps = [
        {
            "x": np.ascontiguousarray(X[i * BPC : (i + 1) * BPC]),
            "w": W2,
            "bias": bias2,
        }
        for i in range(NCORES)
    ]
    return nc, in_maps


def _gather(results):
    O = np.concatenate([r["out"] for r in results], axis=0)  # [32, 128, 4088]
    O = O.reshape(B, 2, F, LOUT).transpose(0, 3, 2, 1)  # [B, LOUT, F, 2]
    return np.ascontiguousarray(O, dtype=np.float32)


def _run(inputs, trace=False, mm_dt_name=None):
    nc, in_maps = _prepare(inputs, mm_dt_name)
    res = run_bass_kernel_spmd(nc, in_maps, core_ids=list(range(NCORES)), trace=trace)
    return _gather(res.results), res


def kernel(**inputs) -> np.ndarray:
    out, _ = _run(inputs, trace=False)
    return out

